# revision 15
# baseline (speedup 1.0000x reference)
"""Gated multi-head self-attention on 8 Trainium2 NeuronCores via Bass/Tile.

Problem: B=2, S=2048, E=1024, H=16, D=64, zero additive mask, gate=ones.
Sharding: core c handles batch b=c//4 and heads [4*(c%4), 4*(c%4)+4).
Each core computes its 4 heads' gated attention partial sum [S, E] in
bf16; the host upcasts and adds the 4 partials per batch.

v2 layout (per core, all bf16 matmuls, fp32 PSUM accumulate):
  xt   [E, S]        X^T for this batch (host pre-transposed + bf16 cast)
  wq   [E, 256]      per-head Wq/sqrt(D) stacked on columns (hd = h*64+d)
  wk   [E, 256]      Wk stacked
  wv   [E, 256]      Wv stacked
  wo   [256, E]      Wo stacked on rows, pre-scaled by eff_gate/denom
  mask [128, S/128]  additive mask column-major by t-chunk
  out  [S, E] bf16   partial output

The 4 heads form 2 pairs. Q^T/K^T keep the natural projection layout
(head-even dims in partitions 0:64, head-odd in 64:128); score tiles
for both heads of a pair are produced CONCURRENTLY by two K=64
row-tiled matmuls (tile_position rows 0/64, separate PSUM slots), so
the PE never pays the duplicated-row 2x score cost.

Steady-state t-loop per (s-block, pair): ACT exps even/odd score tiles
back-to-back ([128,1024] each); the PE interleaves PV (65-col MMs into
a 3-bank paired context accumulator with a free ones-column denominator),
next-t score pairs, and fine-grained injected projection/O-projection
chunks through a single aux PSUM bank. Normalize muls run on the Pool
engine; tail transposes alternate the SP and ACT DMA queues.
"""

import math
import os

import numpy as np

B = 2
S = 2048
E = 1024
H = 16
D = 64
P = 128
GATE_EPS = 1e-4
N_CORES = 8
NH = 4          # heads per core
NPAIR = 2       # head pairs per core
HDC = NH * D    # 256 stacked head-dim columns per core
SBLK = 1024
NSB = S // SBLK     # 2 s-blocks
TCH = S // P        # 16 t-chunks
KT_E = E // P       # 8 k-tiles over the embedding contraction
NSC = SBLK // P     # 8 s-chunks per s-block

_BUILT = {}


def _build(debug=False):
    """Build the single-core Bass program (same program on all 8 cores)."""
    import concourse.bacc as bacc
    import concourse.mybir as mybir
    import concourse.tile as tile
    from contextlib import ExitStack

    bf16 = mybir.dt.bfloat16
    fp32 = mybir.dt.float32
    AF = mybir.ActivationFunctionType

    nc = bacc.Bacc()
    xt = nc.dram_tensor("xt", [E, S], bf16, kind="ExternalInput")
    wq = nc.dram_tensor("wq", [E, HDC], bf16, kind="ExternalInput")
    wk = nc.dram_tensor("wk", [E, HDC], bf16, kind="ExternalInput")
    wv = nc.dram_tensor("wv", [E, HDC], bf16, kind="ExternalInput")
    wo = nc.dram_tensor("wo", [HDC, E], bf16, kind="ExternalInput")
    mask = nc.dram_tensor("mask", [P, TCH], fp32, kind="ExternalInput")
    out = nc.dram_tensor("out", [S, E], bf16, kind="ExternalOutput")
    if debug:
        dbg_qt = nc.dram_tensor("dbg_qt", [P, NPAIR, S], bf16, kind="ExternalOutput")
        dbg_kt = nc.dram_tensor("dbg_kt", [P, NPAIR, S], bf16, kind="ExternalOutput")
        dbg_vt = nc.dram_tensor("dbg_vt", [P, TCH, NH, D + 1], bf16, kind="ExternalOutput")
        dbg_ct = nc.dram_tensor("dbg_ct", [P, HDC // P, S], bf16, kind="ExternalOutput")
        dbg_ct_b = nc.dram_tensor("dbg_ct_b", [P, HDC // P, S], bf16, kind="ExternalOutput")
        dbg_ct_c = nc.dram_tensor("dbg_ct_c", [P, HDC // P, S], bf16, kind="ExternalOutput")
        dbg_cn = nc.dram_tensor("dbg_cn", [P, NSC, P], bf16, kind="ExternalOutput")

    with tile.TileContext(nc) as tc, ExitStack() as ctx:
        const = ctx.enter_context(tc.tile_pool(name="const", bufs=1))
        xt_sb = const.tile([P, KT_E, S], bf16, tag="xt")
        wq_sb = const.tile([P, KT_E, HDC], bf16, tag="wq")
        wk_sb = const.tile([P, KT_E, HDC], bf16, tag="wk")
        wv_sb = const.tile([P, KT_E, HDC], bf16, tag="wv")
        wo_sb = const.tile([P, HDC // P, E], bf16, tag="wo")
        mask_sb = const.tile([P, TCH], fp32, tag="mask")
        # per-pair Q^T/K^T: head-even dims in partitions 0:64, head-odd
        # in 64:128 (the natural [hd, s] projection layout)
        qt_sb = const.tile([P, NPAIR, S], bf16, tag="qt")
        kt_sb = const.tile([P, NPAIR, S], bf16, tag="kt")
        # bf16 V with an appended ones column for softmax denominators
        vt_sb = const.tile([P, TCH, NH, D + 1], bf16, tag="vt")
        ct_sb = const.tile([P, HDC // P, S], bf16, tag="ct")
        # normalized context staging for the [s,hd]->[hd,s] DMA transposes
        cn_buf = const.tile([P, NSC, P], bf16, tag="cn_buf")

        # DMA order = first-needed-first: weights + s-block-0 of xt, then
        # the rest, so the first K/Q projections can start ~6us in.
        nc.sync.dma_start(wk_sb[:], wk.rearrange("(ko p) n -> p ko n", p=P))
        nc.sync.dma_start(wq_sb[:], wq.rearrange("(ko p) n -> p ko n", p=P))
        nc.sync.dma_start(mask_sb[:], mask[:])
        xt_r = xt.rearrange("(ko p) s -> p ko s", p=P)
        for k in range(KT_E):
            nc.sync.dma_start(xt_sb[:, k, 0:SBLK], xt_r[:, k, 0:SBLK])
        nc.sync.dma_start(wv_sb[:], wv.rearrange("(ko p) n -> p ko n", p=P))
        for k in range(KT_E):
            nc.sync.dma_start(xt_sb[:, k, SBLK:S], xt_r[:, k, SBLK:S])
        nc.sync.dma_start(wo_sb[:], wo.rearrange("(kt p) e -> p kt e", p=P))
        nc.vector.memset(vt_sb[:, :, :, D : D + 1], 1.0)

        with tc.tile_pool(name="sc_psum", bufs=1, space="PSUM") as sc_pool, \
             tc.tile_pool(name="cacc_psum", bufs=1, space="PSUM") as cacc_pool, \
             tc.tile_pool(name="aux_psum", bufs=1, space="PSUM") as aux_pool, \
             tc.tile_pool(name="pt_pool", bufs=6) as pt_pool, \
             tc.tile_pool(name="norm_pool", bufs=4) as norm_pool, \
             tc.tile_pool(name="out_pool", bufs=3) as out_pool:

            # PSUM: 2 score slots (2 banks each) + 3 cacc banks + 1 aux = 8
            slot_e = sc_pool.tile([P, SBLK], fp32, tag="slotE")
            slot_o = sc_pool.tile([P, SBLK], fp32, tag="slotO")
            slot = [slot_e, slot_o]
            cacc0 = cacc_pool.tile([P, NSC - 1, D + 1], fp32, tag="cacc0")
            cacc1 = cacc_pool.tile([P, NSC - 1, D + 1], fp32, tag="cacc1")
            cacc2 = cacc_pool.tile([P, 2, D + 1], fp32, tag="cacc2")
            aux = aux_pool.tile([P, 512], fp32, tag="aux")

            def cacc_ap(par, scnk):
                if scnk < NSC - 1:
                    return (cacc0 if par == 0 else cacc1)[:, scnk, :]
                return cacc2[:, par, :]

            # ---------------- injected work units ----------------
            # Each unit is a closure emitting <= ~1us of PE work; the aux
            # bank serializes units of one chunk, Tile handles hazards.

            def qk_unit(w_sb, dst, pair, s0, klo, khi, ps=None):
                """Half of a [128 hd, 512 s] Q^T/K^T projection chunk."""
                if ps is None:
                    ps = aux
                for k in range(klo, khi):
                    nc.tensor.matmul(
                        ps[:, 0:512],
                        lhsT=w_sb[:, k, pair * P : (pair + 1) * P],
                        rhs=xt_sb[:, k, s0 : s0 + 512],
                        start=(k == 0),
                        stop=(k == KT_E - 1),
                        skip_group_check=True,
                    )
                if khi == KT_E:
                    nc.vector.tensor_copy(
                        out=dst[:, pair, s0 : s0 + 512], in_=ps[:, 0:512]
                    )

            def v_unit(t):
                """Project V for t-chunk t into the [t, h, 65] layout."""
                for k in range(KT_E):
                    nc.tensor.matmul(
                        aux[:, 0:HDC],
                        lhsT=xt_sb[:, k, t * P : (t + 1) * P],
                        rhs=wv_sb[:, k, :],
                        start=(k == 0),
                        stop=(k == KT_E - 1),
                        skip_group_check=True,
                    )
                nc.vector.tensor_copy(
                    out=vt_sb[:, t, :, 0:D],
                    in_=aux[:, 0:HDC].rearrange("p (h d) -> p h d", d=D),
                )

            def o_unit(schunk, half, ob):
                """Half of one [128 s, 1024 e] output-projection chunk."""
                for kt2 in range(HDC // P):
                    nc.tensor.matmul(
                        aux[:],
                        lhsT=ct_sb[:, kt2, schunk * P : (schunk + 1) * P],
                        rhs=wo_sb[:, kt2, half * 512 : half * 512 + 512],
                        start=(kt2 == 0),
                        stop=(kt2 == HDC // P - 1),
                        skip_group_check=True,
                    )
                nc.vector.tensor_copy(
                    out=ob[:, half * 512 : half * 512 + 512], in_=aux[:]
                )
                nc.sync.dma_start(
                    out[schunk * P : (schunk + 1) * P, half * 512 : half * 512 + 512],
                    ob[:, half * 512 : half * 512 + 512],
                )

            def emit_scores(pair, sb, t):
                """Both heads' [128 t, SBLK s] score tiles, row-tiled K=64."""
                for par in range(2):
                    lo, hi = (0, D) if par == 0 else (D, P)
                    for sc0 in range(0, SBLK, 512):
                        nc.tensor.matmul(
                            slot[par][:, sc0 : sc0 + 512],
                            lhsT=kt_sb[lo:hi, pair, t * P : (t + 1) * P],
                            rhs=qt_sb[lo:hi, pair, sb * SBLK + sc0 : sb * SBLK + sc0 + 512],
                            start=True,
                            stop=True,
                        )

            def emit_exp(pair, t, par, dst):
                nc.scalar.activation(
                    dst[:],
                    slot[par][:],
                    AF.Exp,
                    bias=mask_sb[:, t : t + 1],
                    scale=1.0,
                )

            def emit_pv(pair, t, par, ptile):
                h = 2 * pair + par
                for scnk in range(NSC):
                    nc.tensor.matmul(
                        cacc_ap(par, scnk),
                        lhsT=ptile[:, scnk * P : (scnk + 1) * P],
                        rhs=vt_sb[:, t, h, :],
                        start=False,
                        stop=(t == TCH - 1),
                        skip_group_check=True,
                    )

            def memset_cacc(par):
                nc.vector.memset((cacc0 if par == 0 else cacc1)[:], 0.0)
                nc.vector.memset(cacc2[:, par, :], 0.0)

            def normalize(pair, sb, par, scnk):
                ca = cacc_ap(par, scnk)
                recip = norm_pool.tile([P, 1], fp32, tag="recip")
                nc.vector.reciprocal(recip[:], ca[:, D : D + 1])
                nc.vector.tensor_scalar_mul(
                    cn_buf[:, scnk, par * D : par * D + D],
                    ca[:, 0:D],
                    recip[:],
                )

            def flush_ct(pair, sb, scnk, engine=None):
                eng = engine if engine is not None else nc.sync
                eng.dma_start_transpose(
                    ct_sb[:, pair, sb * SBLK + scnk * P : sb * SBLK + (scnk + 1) * P],
                    cn_buf[:, scnk, :],
                )

            # Injected units, popped two per t-iteration (one before the
            # next-t scores, one after). Ordering is deadline-driven:
            # scores(t+1) at iter t reads kt cols for t-chunk t+1 and (at
            # the loop boundary) the next pair's qt — every producing CAST
            # must be POPPED before that emit. A chunk's two halves share
            # the aux accumulation, so they must stay adjacent pops.
            def qk_pair(w_sb, dst, pair, s0):
                return [
                    lambda: qk_unit(w_sb, dst, pair, s0, 0, 4),
                    lambda: qk_unit(w_sb, dst, pair, s0, 4, KT_E),
                ]

            pending = []
            vs = list(range(2, TCH))
            loop_a_chunks = (
                [(wk_sb, kt_sb, 0, SBLK), (wk_sb, kt_sb, 0, SBLK + 512)]
                + [(wk_sb, kt_sb, 1, s0) for s0 in range(0, S, 512)]
                + [(wq_sb, qt_sb, 1, 0), (wq_sb, qt_sb, 1, 512)]
            )
            for ch in loop_a_chunks:
                pending += qk_pair(*ch)
                for _ in range(2):
                    if vs:
                        t_ = vs.pop(0)
                        pending.append(lambda t_=t_: v_unit(t_))
            # second-s-block Q^T for both pairs, due by the sb-1 loops
            for pair in range(NPAIR):
                for s0 in (SBLK, SBLK + 512):
                    pending += qk_pair(wq_sb, qt_sb, pair, s0)

            # ---- upfront: what the (sb 0, pair 0) t-loop needs to start
            qk_unit(wk_sb, kt_sb, 0, 0, 0, KT_E)
            qk_unit(wk_sb, kt_sb, 0, 512, 0, KT_E)
            qk_unit(wq_sb, qt_sb, 0, 0, 0, KT_E)
            qk_unit(wq_sb, qt_sb, 0, 512, 0, KT_E)
            memset_cacc(0)
            memset_cacc(1)

            first = True
            for sb in range(NSB):
                for pair in range(NPAIR):
                    if first:
                        # v units for t-chunks 0,1 ride inside the ramp
                        emit_scores(0, 0, 0)
                        v_unit(0)
                        v_unit(1)
                    for t in range(TCH):
                        # ACT order: exp_even(t), exp_odd(t)
                        pt_e = pt_pool.tile([P, SBLK], bf16, tag="pt")
                        pt_o = pt_pool.tile([P, SBLK], bf16, tag="pt")
                        emit_exp(pair, t, 0, pt_e)
                        emit_exp(pair, t, 1, pt_o)
                        # PE order: PV_even(t) | inject | scores(t+1) |
                        # inject | PV_odd(t)
                        emit_pv(pair, t, 0, pt_e)
                        if pending:
                            pending.pop(0)()
                        if t + 1 < TCH:
                            emit_scores(pair, sb, t + 1)
                        elif not (sb == NSB - 1 and pair == NPAIR - 1):
                            nsb, npair = (sb, 1) if pair == 0 else (sb + 1, 0)
                            emit_scores(npair, nsb, 0)
                        if pending:
                            pending.pop(0)()
                        emit_pv(pair, t, 1, pt_o)
                    first = False
                    # ---- normalize + ct flush; memsets free cacc for the
                    # next pair's PVs as soon as its parity is drained
                    last = sb == NSB - 1 and pair == NPAIR - 1
                    for scnk in range(NSC):
                        normalize(pair, sb, 0, scnk)
                    for scnk in range(NSC):
                        normalize(pair, sb, 1, scnk)
                        if not last:
                            flush_ct(pair, sb, scnk)
                        else:
                            flush_ct(pair, sb, scnk,
                                     engine=(nc.sync if scnk % 2 == 0 else nc.scalar))
                    if not last:
                        memset_cacc(0)
                        memset_cacc(1)
                    if debug and sb == 0 and pair == NPAIR - 1:
                        nc.sync.dma_start(dbg_ct_b[:], ct_sb[:])
                    if debug and sb == NSB - 1 and pair == 0:
                        nc.sync.dma_start(dbg_ct_c[:], ct_sb[:])
                    if sb == 0 and pair == NPAIR - 1:
                        # queue s-block 0's O chunks behind the remaining
                        # projection units
                        for schunk in range(NSC):
                            ob = out_pool.tile([P, E], bf16, tag="ob")
                            for half in range(2):
                                pending.append(
                                    lambda schunk=schunk, half=half, ob=ob: o_unit(
                                        schunk, half, ob
                                    )
                                )
                    if last:
                        # tail: final s-block's O chunks chase the flushes
                        for schunk in range(NSC):
                            ob = out_pool.tile([P, E], bf16, tag="ob")
                            for half in range(2):
                                o_unit(NSC + schunk, half, ob)
            # drain any injections that didn't fit the loops
            while pending:
                pending.pop(0)()
            if debug:
                nc.sync.dma_start(dbg_qt[:], qt_sb[:])
                nc.sync.dma_start(dbg_kt[:], kt_sb[:])
                nc.sync.dma_start(dbg_vt[:], vt_sb[:])
                nc.sync.dma_start(dbg_ct[:], ct_sb[:])
                nc.sync.dma_start(dbg_cn[:], cn_buf[:])

    nc.compile()
    return nc


def _get_built():
    if "nc" not in _BUILT:
        _BUILT["nc"] = _build()
    return _BUILT["nc"]


def _host_prep(hidden_states, attention_mask, W_q, W_k, W_v, W_o, gate):
    import ml_dtypes

    bf16 = ml_dtypes.bfloat16
    hs = np.asarray(hidden_states, dtype=np.float32)
    am = np.asarray(attention_mask, dtype=np.float32)
    W_q = np.asarray(W_q, dtype=np.float32)
    W_k = np.asarray(W_k, dtype=np.float32)
    W_v = np.asarray(W_v, dtype=np.float32)
    W_o = np.asarray(W_o, dtype=np.float32)
    gate = np.asarray(gate, dtype=np.float32)

    eff_gate = np.where(gate >= GATE_EPS, gate, 0.0)
    active = float(np.sum(gate > GATE_EPS))
    denom = max(1.0, active / H) if active > 0 else 1.0

    scale = 1.0 / math.sqrt(D)
    # [H, E, D] -> [E, H*D] head-stacked
    wq_all = np.ascontiguousarray((W_q * scale).transpose(1, 0, 2).reshape(E, H * D)).astype(bf16)
    wk_all = np.ascontiguousarray(W_k.transpose(1, 0, 2).reshape(E, H * D)).astype(bf16)
    wv_all = np.ascontiguousarray(W_v.transpose(1, 0, 2).reshape(E, H * D)).astype(bf16)
    wo_scaled = (W_o * (eff_gate / denom)[:, None, None]).reshape(H * D, E).astype(bf16)

    in_maps = []
    for c in range(N_CORES):
        b = c // 4
        g = c % 4
        hd0 = g * NH * D
        xt_c = np.ascontiguousarray(hs[b].T).astype(bf16)  # [E, S]
        mask_c = np.ascontiguousarray(
            am[b, 0, 0, :].reshape(TCH, P).T
        ).astype(np.float32)  # [128, TCH]
        in_maps.append(
            {
                "xt": xt_c,
                "wq": np.ascontiguousarray(wq_all[:, hd0 : hd0 + HDC]),
                "wk": np.ascontiguousarray(wk_all[:, hd0 : hd0 + HDC]),
                "wv": np.ascontiguousarray(wv_all[:, hd0 : hd0 + HDC]),
                "wo": np.ascontiguousarray(wo_scaled[hd0 : hd0 + HDC, :]),
                "mask": mask_c,
            }
        )
    return in_maps


LAST_RESULTS = None


def _ensure_ntff_hook():
    """Install the antenv.axon_hooks shim + ctypes NTFF hook if absent."""
    import sys
    import types

    try:
        from antenv.axon_hooks import get_axon_ntff_profile_hook  # noqa: F401

        return
    except ImportError:
        pass
    mod = types.ModuleType("antenv.axon_hooks")
    state = {"hook": None}
    mod.set_axon_ntff_profile_hook = lambda h: state.__setitem__("hook", h)
    mod.get_axon_ntff_profile_hook = lambda: state["hook"]
    sys.modules["antenv.axon_hooks"] = mod
    try:
        import antenv

        antenv.axon_hooks = mod
    except ImportError:
        pass
    try:
        from trn_agent_boot.trn_boot import _ntff_profile_via_ctypes

        mod.set_axon_ntff_profile_hook(
            _ntff_profile_via_ctypes("/opt/axon/libaxon_pjrt.so")
        )
    except Exception:
        pass


def kernel(hidden_states, attention_mask, W_q, W_k, W_v, W_o, gate):
    global LAST_RESULTS
    from concourse.bass_utils import run_bass_kernel_spmd

    nc = _get_built()
    in_maps = _host_prep(hidden_states, attention_mask, W_q, W_k, W_v, W_o, gate)
    trace = bool(os.environ.get("BASS_TRACE"))
    if trace:
        _ensure_ntff_hook()
    res = run_bass_kernel_spmd(nc, in_maps, core_ids=list(range(N_CORES)), trace=trace)
    LAST_RESULTS = res

    out = np.zeros((B, S, E), dtype=np.float32)
    for c in range(N_CORES):
        out[c // 4] += np.asarray(res.results[c]["out"], dtype=np.float32)
    return out


# revision 16
# speedup vs baseline: 1.0235x; 1.0235x over previous
"""Gated multi-head self-attention on 8 Trainium2 NeuronCores via Bass/Tile.

Problem: B=2, S=2048, E=1024, H=16, D=64, zero additive mask, gate=ones.
Sharding: core c handles batch b=c//4 and heads [4*(c%4), 4*(c%4)+4).
Each core computes its 4 heads' gated attention partial sum [S, E] in
bf16; the host upcasts and adds the 4 partials per batch.

v2 layout (per core, all bf16 matmuls, fp32 PSUM accumulate):
  xt   [E, S]        X^T for this batch (host pre-transposed + bf16 cast)
  wq   [E, 256]      per-head Wq/sqrt(D) stacked on columns (hd = h*64+d)
  wk   [E, 256]      Wk stacked
  wv   [E, 256]      Wv stacked
  wo   [256, E]      Wo stacked on rows, pre-scaled by eff_gate/denom
  mask [128, S/128]  additive mask column-major by t-chunk
  out  [S, E] bf16   partial output

The 4 heads form 2 pairs. Q^T/K^T keep the natural projection layout
(head-even dims in partitions 0:64, head-odd in 64:128); score tiles
for both heads of a pair are produced CONCURRENTLY by two K=64
row-tiled matmuls (tile_position rows 0/64, separate PSUM slots), so
the PE never pays the duplicated-row 2x score cost.

Steady-state t-loop per (s-block, pair): ACT exps even/odd score tiles
back-to-back ([128,1024] each); the PE interleaves PV (65-col MMs into
a 3-bank paired context accumulator with a free ones-column denominator),
next-t score pairs, and fine-grained injected projection/O-projection
chunks through a single aux PSUM bank. Normalize muls run on the Pool
engine; tail transposes alternate the SP and ACT DMA queues.
"""

import math
import os

import numpy as np

B = 2
S = 2048
E = 1024
H = 16
D = 64
P = 128
GATE_EPS = 1e-4
N_CORES = 8
NH = 4          # heads per core
NPAIR = 2       # head pairs per core
HDC = NH * D    # 256 stacked head-dim columns per core
SBLK = 1024
NSB = S // SBLK     # 2 s-blocks
TCH = S // P        # 16 t-chunks
KT_E = E // P       # 8 k-tiles over the embedding contraction
NSC = SBLK // P     # 8 s-chunks per s-block

_BUILT = {}


def _build(debug=False):
    """Build the single-core Bass program (same program on all 8 cores)."""
    import concourse.bacc as bacc
    import concourse.mybir as mybir
    import concourse.tile as tile
    from contextlib import ExitStack

    bf16 = mybir.dt.bfloat16
    fp32 = mybir.dt.float32
    AF = mybir.ActivationFunctionType

    nc = bacc.Bacc()
    xt = nc.dram_tensor("xt", [E, S], bf16, kind="ExternalInput")
    wq = nc.dram_tensor("wq", [E, HDC], bf16, kind="ExternalInput")
    wk = nc.dram_tensor("wk", [E, HDC], bf16, kind="ExternalInput")
    wv = nc.dram_tensor("wv", [E, HDC], bf16, kind="ExternalInput")
    wo = nc.dram_tensor("wo", [HDC, E], bf16, kind="ExternalInput")
    mask = nc.dram_tensor("mask", [P, TCH], fp32, kind="ExternalInput")
    out = nc.dram_tensor("out", [S, E], bf16, kind="ExternalOutput")
    if debug:
        dbg_qt = nc.dram_tensor("dbg_qt", [P, NPAIR, S], bf16, kind="ExternalOutput")
        dbg_kt = nc.dram_tensor("dbg_kt", [P, NPAIR, S], bf16, kind="ExternalOutput")
        dbg_vt = nc.dram_tensor("dbg_vt", [P, TCH, NH, D + 1], bf16, kind="ExternalOutput")
        dbg_ct = nc.dram_tensor("dbg_ct", [P, HDC // P, S], bf16, kind="ExternalOutput")
        dbg_ct_b = nc.dram_tensor("dbg_ct_b", [P, HDC // P, S], bf16, kind="ExternalOutput")
        dbg_ct_c = nc.dram_tensor("dbg_ct_c", [P, HDC // P, S], bf16, kind="ExternalOutput")
        dbg_cn = nc.dram_tensor("dbg_cn", [P, NSC, P], bf16, kind="ExternalOutput")

    with tile.TileContext(nc) as tc, ExitStack() as ctx:
        const = ctx.enter_context(tc.tile_pool(name="const", bufs=1))
        xt_sb = const.tile([P, KT_E, S], bf16, tag="xt")
        wq_sb = const.tile([P, KT_E, HDC], bf16, tag="wq")
        wk_sb = const.tile([P, KT_E, HDC], bf16, tag="wk")
        wv_sb = const.tile([P, KT_E, HDC], bf16, tag="wv")
        wo_sb = const.tile([P, HDC // P, E], bf16, tag="wo")
        mask_sb = const.tile([P, TCH], fp32, tag="mask")
        # per-pair Q^T/K^T: head-even dims in partitions 0:64, head-odd
        # in 64:128 (the natural [hd, s] projection layout)
        qt_sb = const.tile([P, NPAIR, S], bf16, tag="qt")
        kt_sb = const.tile([P, NPAIR, S], bf16, tag="kt")
        # bf16 V with an appended ones column for softmax denominators
        vt_sb = const.tile([P, TCH, NH, D + 1], bf16, tag="vt")
        ct_sb = const.tile([P, HDC // P, S], bf16, tag="ct")
        # normalized context staging for the [s,hd]->[hd,s] DMA transposes
        cn_buf = const.tile([P, NSC, P], bf16, tag="cn_buf")

        # DMA order = first-needed-first: weights + s-block-0 of xt, then
        # the rest, so the first K/Q projections can start ~6us in.
        nc.sync.dma_start(wk_sb[:], wk.rearrange("(ko p) n -> p ko n", p=P))
        nc.sync.dma_start(wq_sb[:], wq.rearrange("(ko p) n -> p ko n", p=P))
        nc.sync.dma_start(mask_sb[:], mask[:])
        xt_r = xt.rearrange("(ko p) s -> p ko s", p=P)
        for k in range(KT_E):
            nc.sync.dma_start(xt_sb[:, k, 0:SBLK], xt_r[:, k, 0:SBLK])
        nc.sync.dma_start(wv_sb[:], wv.rearrange("(ko p) n -> p ko n", p=P))
        for k in range(KT_E):
            nc.sync.dma_start(xt_sb[:, k, SBLK:S], xt_r[:, k, SBLK:S])
        nc.sync.dma_start(wo_sb[:], wo.rearrange("(kt p) e -> p kt e", p=P))
        nc.vector.memset(vt_sb[:, :, :, D : D + 1], 1.0)

        with tc.tile_pool(name="sc_psum", bufs=1, space="PSUM") as sc_pool, \
             tc.tile_pool(name="cacc_psum", bufs=1, space="PSUM") as cacc_pool, \
             tc.tile_pool(name="aux_psum", bufs=1, space="PSUM") as aux_pool, \
             tc.tile_pool(name="pt_pool", bufs=6) as pt_pool, \
             tc.tile_pool(name="norm_pool", bufs=4) as norm_pool, \
             tc.tile_pool(name="out_pool", bufs=3) as out_pool:

            # PSUM: 2 score slots (2 banks each) + 3 cacc banks + 1 aux = 8
            slot_e = sc_pool.tile([P, SBLK], fp32, tag="slotE")
            slot_o = sc_pool.tile([P, SBLK], fp32, tag="slotO")
            slot = [slot_e, slot_o]
            cacc0 = cacc_pool.tile([P, NSC - 1, D + 1], fp32, tag="cacc0")
            cacc1 = cacc_pool.tile([P, NSC - 1, D + 1], fp32, tag="cacc1")
            cacc2 = cacc_pool.tile([P, 2, D + 1], fp32, tag="cacc2")
            aux = aux_pool.tile([P, 512], fp32, tag="aux")

            def cacc_ap(par, scnk):
                if scnk < NSC - 1:
                    return (cacc0 if par == 0 else cacc1)[:, scnk, :]
                return cacc2[:, par, :]

            # ---------------- injected work units ----------------
            # Each unit is a closure emitting <= ~1us of PE work; the aux
            # bank serializes units of one chunk, Tile handles hazards.

            def qk_unit(w_sb, dst, pair, s0, klo, khi, ps=None):
                """Half of a [128 hd, 512 s] Q^T/K^T projection chunk."""
                if ps is None:
                    ps = aux
                for k in range(klo, khi):
                    nc.tensor.matmul(
                        ps[:, 0:512],
                        lhsT=w_sb[:, k, pair * P : (pair + 1) * P],
                        rhs=xt_sb[:, k, s0 : s0 + 512],
                        start=(k == 0),
                        stop=(k == KT_E - 1),
                        skip_group_check=True,
                    )
                if khi == KT_E:
                    nc.vector.tensor_copy(
                        out=dst[:, pair, s0 : s0 + 512], in_=ps[:, 0:512]
                    )

            def v_unit(t):
                """Project V for t-chunk t into the [t, h, 65] layout."""
                for k in range(KT_E):
                    nc.tensor.matmul(
                        aux[:, 0:HDC],
                        lhsT=xt_sb[:, k, t * P : (t + 1) * P],
                        rhs=wv_sb[:, k, :],
                        start=(k == 0),
                        stop=(k == KT_E - 1),
                        skip_group_check=True,
                    )
                nc.vector.tensor_copy(
                    out=vt_sb[:, t, :, 0:D],
                    in_=aux[:, 0:HDC].rearrange("p (h d) -> p h d", d=D),
                )

            def o_unit(schunk, half, ob):
                """Half of one [128 s, 1024 e] output-projection chunk."""
                for kt2 in range(HDC // P):
                    nc.tensor.matmul(
                        aux[:],
                        lhsT=ct_sb[:, kt2, schunk * P : (schunk + 1) * P],
                        rhs=wo_sb[:, kt2, half * 512 : half * 512 + 512],
                        start=(kt2 == 0),
                        stop=(kt2 == HDC // P - 1),
                        skip_group_check=True,
                    )
                nc.vector.tensor_copy(
                    out=ob[:, half * 512 : half * 512 + 512], in_=aux[:]
                )
                nc.sync.dma_start(
                    out[schunk * P : (schunk + 1) * P, half * 512 : half * 512 + 512],
                    ob[:, half * 512 : half * 512 + 512],
                )

            def emit_scores(pair, sb, t):
                """Both heads' [128 t, SBLK s] score tiles, row-tiled K=64."""
                for par in range(2):
                    lo, hi = (0, D) if par == 0 else (D, P)
                    for sc0 in range(0, SBLK, 512):
                        nc.tensor.matmul(
                            slot[par][:, sc0 : sc0 + 512],
                            lhsT=kt_sb[lo:hi, pair, t * P : (t + 1) * P],
                            rhs=qt_sb[lo:hi, pair, sb * SBLK + sc0 : sb * SBLK + sc0 + 512],
                            start=True,
                            stop=True,
                        )

            def emit_exp(pair, t, par, dst):
                nc.scalar.activation(
                    dst[:],
                    slot[par][:],
                    AF.Exp,
                    bias=mask_sb[:, t : t + 1],
                    scale=1.0,
                )

            def emit_pv(pair, t, par, ptile):
                h = 2 * pair + par
                for scnk in range(NSC):
                    nc.tensor.matmul(
                        cacc_ap(par, scnk),
                        lhsT=ptile[:, scnk * P : (scnk + 1) * P],
                        rhs=vt_sb[:, t, h, :],
                        start=False,
                        stop=(t == TCH - 1),
                        skip_group_check=True,
                    )

            def memset_cacc(par):
                nc.vector.memset((cacc0 if par == 0 else cacc1)[:], 0.0)
                nc.vector.memset(cacc2[:, par, :], 0.0)

            def normalize(pair, sb, par, scnk):
                ca = cacc_ap(par, scnk)
                recip = norm_pool.tile([P, 1], fp32, tag="recip")
                nc.vector.reciprocal(recip[:], ca[:, D : D + 1])
                nc.vector.tensor_scalar_mul(
                    cn_buf[:, scnk, par * D : par * D + D],
                    ca[:, 0:D],
                    recip[:],
                )

            def flush_ct(pair, sb, scnk, engine=None):
                eng = engine if engine is not None else nc.sync
                eng.dma_start_transpose(
                    ct_sb[:, pair, sb * SBLK + scnk * P : sb * SBLK + (scnk + 1) * P],
                    cn_buf[:, scnk, :],
                )

            # Injected units, popped two per t-iteration (one before the
            # next-t scores, one after). Ordering is deadline-driven:
            # scores(t+1) at iter t reads kt cols for t-chunk t+1 and (at
            # the loop boundary) the next pair's qt — every producing CAST
            # must be POPPED before that emit. A chunk's two halves share
            # the aux accumulation, so they must stay adjacent pops.
            def qk_pair(w_sb, dst, pair, s0):
                return [
                    lambda: qk_unit(w_sb, dst, pair, s0, 0, 4),
                    lambda: qk_unit(w_sb, dst, pair, s0, 4, KT_E),
                ]

            pending = []
            vs = list(range(2, TCH))
            loop_a_chunks = (
                [(wk_sb, kt_sb, 0, SBLK), (wk_sb, kt_sb, 0, SBLK + 512)]
                + [(wk_sb, kt_sb, 1, s0) for s0 in range(0, S, 512)]
                + [(wq_sb, qt_sb, 1, 0), (wq_sb, qt_sb, 1, 512)]
            )
            for ch in loop_a_chunks:
                pending += qk_pair(*ch)
                for _ in range(2):
                    if vs:
                        t_ = vs.pop(0)
                        pending.append(lambda t_=t_: v_unit(t_))
            # second-s-block Q^T for both pairs, due by the sb-1 loops
            for pair in range(NPAIR):
                for s0 in (SBLK, SBLK + 512):
                    pending += qk_pair(wq_sb, qt_sb, pair, s0)

            # ---- upfront: what the (sb 0, pair 0) t-loop needs to start
            qk_unit(wk_sb, kt_sb, 0, 0, 0, KT_E)
            qk_unit(wk_sb, kt_sb, 0, 512, 0, KT_E)
            qk_unit(wq_sb, qt_sb, 0, 0, 0, KT_E)
            qk_unit(wq_sb, qt_sb, 0, 512, 0, KT_E)
            memset_cacc(0)
            memset_cacc(1)

            first = True
            for sb in range(NSB):
                for pair in range(NPAIR):
                    if first:
                        # v units for t-chunks 0,1 ride inside the ramp
                        emit_scores(0, 0, 0)
                        v_unit(0)
                        v_unit(1)
                    for t in range(TCH):
                        # ACT order: exp_even(t), exp_odd(t)
                        pt_e = pt_pool.tile([P, SBLK], bf16, tag="pt")
                        pt_o = pt_pool.tile([P, SBLK], bf16, tag="pt")
                        emit_exp(pair, t, 0, pt_e)
                        emit_exp(pair, t, 1, pt_o)
                        # PE order: PV_even(t) | inject | scores(t+1) |
                        # inject | PV_odd(t)
                        emit_pv(pair, t, 0, pt_e)
                        if pending:
                            pending.pop(0)()
                        if t + 1 < TCH:
                            emit_scores(pair, sb, t + 1)
                        elif not (sb == NSB - 1 and pair == NPAIR - 1):
                            nsb, npair = (sb, 1) if pair == 0 else (sb + 1, 0)
                            emit_scores(npair, nsb, 0)
                        if pending:
                            pending.pop(0)()
                        emit_pv(pair, t, 1, pt_o)
                    first = False
                    # ---- normalize + ct flush; memsets free cacc for the
                    # next pair's PVs as soon as its parity is drained
                    last = sb == NSB - 1 and pair == NPAIR - 1
                    for scnk in range(NSC):
                        normalize(pair, sb, 0, scnk)
                    for scnk in range(NSC):
                        normalize(pair, sb, 1, scnk)
                        flush_ct(pair, sb, scnk)
                    if not last:
                        memset_cacc(0)
                        memset_cacc(1)
                    if debug and sb == 0 and pair == NPAIR - 1:
                        nc.sync.dma_start(dbg_ct_b[:], ct_sb[:])
                    if debug and sb == NSB - 1 and pair == 0:
                        nc.sync.dma_start(dbg_ct_c[:], ct_sb[:])
                    if sb == 0 and pair == NPAIR - 1:
                        # queue s-block 0's O chunks behind the remaining
                        # projection units
                        for schunk in range(NSC):
                            ob = out_pool.tile([P, E], bf16, tag="ob")
                            for half in range(2):
                                pending.append(
                                    lambda schunk=schunk, half=half, ob=ob: o_unit(
                                        schunk, half, ob
                                    )
                                )
                    if last:
                        # tail: final s-block's O chunks chase the flushes
                        for schunk in range(NSC):
                            ob = out_pool.tile([P, E], bf16, tag="ob")
                            for half in range(2):
                                o_unit(NSC + schunk, half, ob)
            # drain any injections that didn't fit the loops
            while pending:
                pending.pop(0)()
            if debug:
                nc.sync.dma_start(dbg_qt[:], qt_sb[:])
                nc.sync.dma_start(dbg_kt[:], kt_sb[:])
                nc.sync.dma_start(dbg_vt[:], vt_sb[:])
                nc.sync.dma_start(dbg_ct[:], ct_sb[:])
                nc.sync.dma_start(dbg_cn[:], cn_buf[:])

    nc.compile()
    return nc


def _get_built():
    if "nc" not in _BUILT:
        _BUILT["nc"] = _build()
    return _BUILT["nc"]


def _host_prep(hidden_states, attention_mask, W_q, W_k, W_v, W_o, gate):
    import ml_dtypes

    bf16 = ml_dtypes.bfloat16
    hs = np.asarray(hidden_states, dtype=np.float32)
    am = np.asarray(attention_mask, dtype=np.float32)
    W_q = np.asarray(W_q, dtype=np.float32)
    W_k = np.asarray(W_k, dtype=np.float32)
    W_v = np.asarray(W_v, dtype=np.float32)
    W_o = np.asarray(W_o, dtype=np.float32)
    gate = np.asarray(gate, dtype=np.float32)

    eff_gate = np.where(gate >= GATE_EPS, gate, 0.0)
    active = float(np.sum(gate > GATE_EPS))
    denom = max(1.0, active / H) if active > 0 else 1.0

    scale = 1.0 / math.sqrt(D)
    # [H, E, D] -> [E, H*D] head-stacked
    wq_all = np.ascontiguousarray((W_q * scale).transpose(1, 0, 2).reshape(E, H * D)).astype(bf16)
    wk_all = np.ascontiguousarray(W_k.transpose(1, 0, 2).reshape(E, H * D)).astype(bf16)
    wv_all = np.ascontiguousarray(W_v.transpose(1, 0, 2).reshape(E, H * D)).astype(bf16)
    wo_scaled = (W_o * (eff_gate / denom)[:, None, None]).reshape(H * D, E).astype(bf16)

    in_maps = []
    for c in range(N_CORES):
        b = c // 4
        g = c % 4
        hd0 = g * NH * D
        xt_c = np.ascontiguousarray(hs[b].T).astype(bf16)  # [E, S]
        mask_c = np.ascontiguousarray(
            am[b, 0, 0, :].reshape(TCH, P).T
        ).astype(np.float32)  # [128, TCH]
        in_maps.append(
            {
                "xt": xt_c,
                "wq": np.ascontiguousarray(wq_all[:, hd0 : hd0 + HDC]),
                "wk": np.ascontiguousarray(wk_all[:, hd0 : hd0 + HDC]),
                "wv": np.ascontiguousarray(wv_all[:, hd0 : hd0 + HDC]),
                "wo": np.ascontiguousarray(wo_scaled[hd0 : hd0 + HDC, :]),
                "mask": mask_c,
            }
        )
    return in_maps


LAST_RESULTS = None


def _ensure_ntff_hook():
    """Install the antenv.axon_hooks shim + ctypes NTFF hook if absent."""
    import sys
    import types

    try:
        from antenv.axon_hooks import get_axon_ntff_profile_hook  # noqa: F401

        return
    except ImportError:
        pass
    mod = types.ModuleType("antenv.axon_hooks")
    state = {"hook": None}
    mod.set_axon_ntff_profile_hook = lambda h: state.__setitem__("hook", h)
    mod.get_axon_ntff_profile_hook = lambda: state["hook"]
    sys.modules["antenv.axon_hooks"] = mod
    try:
        import antenv

        antenv.axon_hooks = mod
    except ImportError:
        pass
    try:
        from trn_agent_boot.trn_boot import _ntff_profile_via_ctypes

        mod.set_axon_ntff_profile_hook(
            _ntff_profile_via_ctypes("/opt/axon/libaxon_pjrt.so")
        )
    except Exception:
        pass


def kernel(hidden_states, attention_mask, W_q, W_k, W_v, W_o, gate):
    global LAST_RESULTS
    from concourse.bass_utils import run_bass_kernel_spmd

    nc = _get_built()
    in_maps = _host_prep(hidden_states, attention_mask, W_q, W_k, W_v, W_o, gate)
    trace = bool(os.environ.get("BASS_TRACE"))
    if trace:
        _ensure_ntff_hook()
    res = run_bass_kernel_spmd(nc, in_maps, core_ids=list(range(N_CORES)), trace=trace)
    LAST_RESULTS = res

    out = np.zeros((B, S, E), dtype=np.float32)
    for c in range(N_CORES):
        out[c // 4] += np.asarray(res.results[c]["out"], dtype=np.float32)
    return out


# revision 25
# speedup vs baseline: 1.4566x; 1.4231x over previous
"""Gated multi-head self-attention on 8 Trainium2 NeuronCores via Bass/Tile.

Problem: B=2, S=2048, E=1024, H=16, D=64, zero additive mask, gate=ones.
Sharding: core c handles batch b=c//4 and heads [4*(c%4), 4*(c%4)+4).
Each core computes its 4 heads' gated attention partial sum [S, E]; the
host adds the 4 partials per batch.

Device-side layout (per core, all bf16 matmuls, fp32 PSUM accumulate):
  xt   [E, S]        X^T for this batch (host pre-transposed + bf16 cast)
  wq   [E, 256]      per-head Wq/sqrt(D) stacked on columns (hd = h*64+d)
  wk   [E, 256]      Wk stacked
  wv   [E, 256]      Wv stacked
  wo   [256, E]      Wo stacked on rows, pre-scaled by eff_gate/denom
  mask [128, S/128]  additive mask column-major by t-chunk
  out  [S, E] fp32   partial output

Pipeline: QK^T projections -> per head: scores^T [t,s] tiles (PE),
exp via ScalarE (mask folded in as per-partition bias), PV with an
appended ones-column in V giving softmax denominators for free,
per-partition normalize (DVE), PE transpose back to [hd, s], final
O-projection, DMA out.
"""

import math
import os

import numpy as np

B = 2
S = 2048
E = 1024
H = 16
D = 64
P = 128
GATE_EPS = 1e-4
N_CORES = 8
NH = 4  # heads per core
HDC = NH * D  # 256 stacked head-dim columns per core

_BUILT = {}


def _build(seq_len=S, sblk=None):
    """Build the single-core Bass program (same program on all 8 cores)."""
    import concourse.bacc as bacc
    import concourse.mybir as mybir
    import concourse.tile as tile
    from contextlib import ExitStack

    bf16 = mybir.dt.bfloat16
    fp8 = mybir.dt.float8e4
    fp32 = mybir.dt.float32
    AF = mybir.ActivationFunctionType

    Sl = seq_len
    if sblk is None:
        sblk = min(1024, Sl)
    SBLK = sblk
    NSB = Sl // SBLK
    TCH = Sl // P  # 128-row t-chunks
    TCH2 = TCH // 2  # t-chunk pairs (DoubleRow contracts 256 rows)
    KT_E = E // P  # k-tiles over the embedding contraction
    NSC = SBLK // P  # 128-col s-chunks per s-block
    VP = 68  # fp8 V row padded so the DoubleRow j-step is 16B-aligned

    nc = bacc.Bacc()
    xt = nc.dram_tensor("xt", [E, Sl], bf16, kind="ExternalInput")
    wq = nc.dram_tensor("wq", [E, HDC], bf16, kind="ExternalInput")
    wk = nc.dram_tensor("wk", [E, HDC], bf16, kind="ExternalInput")
    wv = nc.dram_tensor("wv", [E, HDC], bf16, kind="ExternalInput")
    wo = nc.dram_tensor("wo", [HDC, E], bf16, kind="ExternalInput")
    mask = nc.dram_tensor("mask", [P, TCH], fp32, kind="ExternalInput")
    out = nc.dram_tensor("out", [Sl, E], bf16, kind="ExternalOutput")

    with tile.TileContext(nc) as tc, ExitStack() as ctx:
        const = ctx.enter_context(tc.tile_pool(name="const", bufs=1))
        xt_sb = const.tile([P, KT_E, Sl], bf16, tag="xt")
        wq_sb = const.tile([P, KT_E, HDC], bf16, tag="wq")
        wk_sb = const.tile([P, KT_E, HDC], bf16, tag="wk")
        wv_sb = const.tile([P, KT_E, HDC], bf16, tag="wv")
        wo_sb = const.tile([P, HDC // P, E], bf16, tag="wo")
        mask_sb = const.tile([P, TCH], fp32, tag="mask")
        # per-head Q^T/K^T with the 64 head rows duplicated to both
        # partition halves, so consecutive t-chunks can run on PE row
        # tiles T0/T8 concurrently
        qt_sb = const.tile([P, NH, Sl], bf16, tag="qt")
        kt_sb = const.tile([P, NH, Sl], bf16, tag="kt")
        # bf16 V with an appended ones column for softmax denominators
        vt_sb = const.tile([P, TCH, NH, D + 1], bf16, tag="vt")
        ct_sb = const.tile([P, HDC // P, Sl], bf16, tag="ct")
        # normalized per-pair context staging for the final [s,hd]->[hd,s]
        # DMA transposes (head-even cols 0:64, head-odd 64:128)
        cn_buf = const.tile([P, NSB, NSC, P], bf16, tag="cn_buf")

        # first-needed-first: the (head 0, s-block 0) Q/K projections need
        # wq/wk + the first SBLK columns of xt; later columns and wv/wo
        # follow so the first exp can fire ~15us in
        nc.sync.dma_start(wq_sb[:], wq.rearrange("(ko p) n -> p ko n", p=P))
        nc.sync.dma_start(wk_sb[:], wk.rearrange("(ko p) n -> p ko n", p=P))
        nc.sync.dma_start(mask_sb[:], mask[:])
        xt_r = xt.rearrange("(ko p) s -> p ko s", p=P)
        for k in range(KT_E):
            nc.sync.dma_start(xt_sb[:, k, 0:SBLK], xt_r[:, k, 0:SBLK])
        nc.sync.dma_start(wv_sb[:], wv.rearrange("(ko p) n -> p ko n", p=P))
        for k in range(KT_E):
            nc.sync.dma_start(xt_sb[:, k, SBLK:Sl], xt_r[:, k, SBLK:Sl])
        nc.sync.dma_start(wo_sb[:], wo.rearrange("(kt p) e -> p kt e", p=P))
        nc.vector.memset(vt_sb[:, :, :, D : D + 1], 1.0)

        aux_pool = ctx.enter_context(tc.tile_pool(name="aux_psum", bufs=1, space="PSUM"))

        AUXW = max(SBLK, 2 * HDC, E)

        def emit_qk_block(w_sb, dst, ht, blk, pool=None):
            """Project one [128 hd, SBLK s] block of Q^T/K^T and write it
            duplicated into the two per-head partition halves."""
            if pool is None:
                ps = aux_pool.tile([P, AUXW], fp32, tag="aux")
            else:
                ps = pool.tile([P, SBLK], fp32, tag="sc")
            for k in range(KT_E):
                for sc0 in range(0, SBLK, 512):
                    sc1 = min(sc0 + 512, SBLK)
                    nc.tensor.matmul(
                        ps[:, sc0:sc1],
                        lhsT=w_sb[:, k, ht * P : (ht + 1) * P],
                        rhs=xt_sb[:, k, blk * SBLK + sc0 : blk * SBLK + sc1],
                        start=(k == 0),
                        stop=(k == KT_E - 1),
                    )
            sblc = slice(blk * SBLK, (blk + 1) * SBLK)
            h0, h1 = 2 * ht, 2 * ht + 1
            nc.vector.tensor_copy(out=dst[0:D, h0, sblc], in_=ps[0:D, 0:SBLK])
            nc.vector.tensor_copy(out=dst[D:P, h1, sblc], in_=ps[D:P, 0:SBLK])
            # replicate each head's rows into the other partition half
            nc.sync.dma_start(dst[D:P, h0, sblc], dst[0:D, h0, sblc])
            nc.sync.dma_start(dst[0:D, h1, sblc], dst[D:P, h1, sblc])

        def emit_v_pair(tp):
            """Project V for t-chunks (2tp, 2tp+1) into the fp8 pair layout."""
            ps = aux_pool.tile([P, AUXW], fp32, tag="aux")
            for j in range(2):
                t = 2 * tp + j
                for k in range(KT_E):
                    nc.tensor.matmul(
                        ps[:, j * HDC : j * HDC + HDC],
                        lhsT=xt_sb[:, k, t * P : (t + 1) * P],
                        rhs=wv_sb[:, k, :],
                        start=(k == 0),
                        stop=(k == KT_E - 1),
                    )
            nc.vector.tensor_copy(
                out=vt_sb[:, 2 * tp : 2 * tp + 2, :, 0:D],
                in_=ps[:, 0 : 2 * HDC].rearrange("p (j h d) -> p j h d", j=2, d=D),
            )

        # deferred projection work, split into k-halves injected at the
        # head boundary and mid-t-loop so each ~1.7us burst fits inside
        # the exp runway instead of stalling ACT for a full 3.4us block
        def make_halves(w_sb, dst, ht, blk):
            state = {}

            def half(klo, khi):
                if "ps" not in state:
                    ps_half = aux_pool.tile([P, AUXW], fp32, tag="aux")
                    state["ps"] = ps_half
                ps = state["ps"]
                for k in range(klo, khi):
                    for sc0 in range(0, SBLK, 512):
                        sc1 = min(sc0 + 512, SBLK)
                        nc.tensor.matmul(
                            ps[:, sc0:sc1],
                            lhsT=w_sb[:, k, ht * P : (ht + 1) * P],
                            rhs=xt_sb[:, k, blk * SBLK + sc0 : blk * SBLK + sc1],
                            start=(k == 0),
                            stop=(k == KT_E - 1),
                            skip_group_check=True,
                        )
                if khi == KT_E:
                    sblc = slice(blk * SBLK, (blk + 1) * SBLK)
                    h0, h1 = 2 * ht, 2 * ht + 1
                    nc.vector.tensor_copy(out=dst[0:D, h0, sblc], in_=ps[0:D, 0:SBLK])
                    nc.vector.tensor_copy(out=dst[D:P, h1, sblc], in_=ps[D:P, 0:SBLK])
                    nc.sync.dma_start(dst[D:P, h0, sblc], dst[0:D, h0, sblc])
                    nc.sync.dma_start(dst[0:D, h1, sblc], dst[D:P, h1, sblc])

            return [
                (dst, ht, blk, lambda: half(0, KT_E // 2)),
                (dst, ht, blk, lambda: half(KT_E // 2, KT_E)),
            ]

        pending = []
        for ht in range(1, NH // 2):
            for blk in range(NSB):
                pending += make_halves(wk_sb, kt_sb, ht, blk)
        for blk in range(1, NSB):
            pending += make_halves(wq_sb, qt_sb, 0, blk)
        for ht in range(1, NH // 2):
            for blk in range(1, NSB):
                pending += make_halves(wq_sb, qt_sb, ht, blk)

        with tc.tile_pool(name="cacc_psum", bufs=1, space="PSUM") as cacc_pool, \
             tc.tile_pool(name="scores_psum", bufs=2, space="PSUM") as sc_pool, \
             tc.tile_pool(name="pt_pool", bufs=8) as pt_pool, \
             tc.tile_pool(name="norm_pool", bufs=4) as norm_pool, \
             tc.tile_pool(name="out_pool", bufs=3) as out_pool:

            # ---- upfront projections: what (s-block 0, head 0) needs.
            # Scores for any s-block run over ALL t, so K^T must be complete
            # for a head-tile before its first use; Q^T is s-block-local.
            # Alternate aux/scores PSUM slots (scores is idle during the DMA
            # ramp) so the blocks pipeline instead of serializing.
            up = (
                [(wq_sb, qt_sb, 0, 0)]
                + [(wk_sb, kt_sb, 0, blk) for blk in range(NSB)]
                + [(wq_sb, qt_sb, ht, 0) for ht in range(1, NH // 2)]
            )
            for i, (w_sb_, dst_, ht_, blk_) in enumerate(up):
                emit_qk_block(w_sb_, dst_, ht_, blk_, pool=(sc_pool if i % 2 else None))
            def emit_o_chunk(schunk, use_sc=False):
                if use_sc:
                    po = sc_pool.tile([P, SBLK], fp32, tag="sc")
                else:
                    po = aux_pool.tile([P, AUXW], fp32, tag="aux")
                for kt2 in range(HDC // P):
                    for ec in range(E // 512):
                        nc.tensor.matmul(
                            po[:, ec * 512 : (ec + 1) * 512],
                            lhsT=ct_sb[:, kt2, schunk * P : (schunk + 1) * P],
                            rhs=wo_sb[:, kt2, ec * 512 : (ec + 1) * 512],
                            start=(kt2 == 0),
                            stop=(kt2 == HDC // P - 1),
                        )
                ob = out_pool.tile([P, E], bf16, tag="ob")
                nc.vector.tensor_copy(out=ob[:, 0:512], in_=po[:, 0:512])
                nc.vector.tensor_copy(out=ob[:, 512:E], in_=po[:, 512:E])
                nc.sync.dma_start(out[schunk * P : (schunk + 1) * P, :], ob[:])

            o_ready = []
            for sb in range(NSB):
                for h in range(NH):
                    pr = h // 2
                    # hoist this head's first two score tiles above the
                    # boundary burst: ACT gets two fresh tiles to exp while
                    # the PE chews the burst, doubling its runway. Illegal
                    # only when this head's own kt/qt blocks are still queued.
                    unsafe = any(
                        (dst_ is kt_sb and ht_ == h // 2 and blk_ == 0)
                        or (dst_ is qt_sb and ht_ == h // 2 and blk_ == sb)
                        for (dst_, ht_, blk_, _fn) in pending
                    )
                    prescored = {}
                    if not (h == 0 and sb == 0) and not unsafe:
                        for t in (0, 1):
                            ps_t = sc_pool.tile([P, SBLK], fp32, tag="sc")
                            for sc0 in range(0, SBLK, 512):
                                sc1 = min(sc0 + 512, SBLK)
                                nc.tensor.matmul(
                                    ps_t[:, sc0:sc1],
                                    lhsT=kt_sb[:, h, t * P : (t + 1) * P],
                                    rhs=qt_sb[:, h, sb * SBLK + sc0 : sb * SBLK + sc1],
                                    start=True,
                                    stop=True,
                                )
                            prescored[t] = ps_t
                    if not (h == 0 and sb == 0):
                        # one ~1.7us half-burst here; its sibling (or an O
                        # chunk) lands at the mid-t-loop site instead of
                        # doubling up into a 3.4us ACT stall
                        if pending:
                            pending.pop(0)[3]()
                        elif o_ready:
                            emit_o_chunk(o_ready.pop(0))
                    # fp32 PSUM context accumulators [s, 65]; explicit zero +
                    # start=False (PSUM start=True zeroes a whole 2KB bank)
                    cacc_a = cacc_pool.tile([P, NSC // 2, D + 1], fp32, tag="cacc_a")
                    cacc_b = cacc_pool.tile([P, NSC - NSC // 2, D + 1], fp32, tag="cacc_b")
                    nc.vector.memset(cacc_a[:], 0.0)
                    nc.vector.memset(cacc_b[:], 0.0)

                    def cacc_ap(scnk):
                        if scnk < NSC // 2:
                            return cacc_a[:, scnk, :]
                        return cacc_b[:, scnk - NSC // 2, :]

                    for tp in range(TCH2):
                        if tp == 3 and not (h == 0 and sb == 0):
                            # mid-loop injection site: the sibling half of
                            # the boundary burst (CASTs land before the
                            # t=8 scores that may consume them)
                            if pending:
                                pending.pop(0)[3]()
                            elif o_ready:
                                emit_o_chunk(o_ready.pop(0))
                        # both t-chunks' scores + exps first: PVs and the V
                        # projection are ACT-independent sinks, so feeding
                        # ScalarE two tiles back-to-back keeps it streaming
                        # while the PE works through the sinks
                        pts = {}
                        for j in range(2):
                            t = 2 * tp + j
                            if t in prescored:
                                sc_ps = prescored.pop(t)
                            else:
                                # full-128 contraction over the duplicated head
                                # rows (host folds the x0.5 compensation into
                                # wq); keeps the PE in one tiling mode with
                                # back-to-back MMs
                                sc_ps = sc_pool.tile([P, SBLK], fp32, tag="sc")
                                for sc0 in range(0, SBLK, 512):
                                    sc1 = min(sc0 + 512, SBLK)
                                    nc.tensor.matmul(
                                        sc_ps[:, sc0:sc1],
                                        lhsT=kt_sb[:, h, t * P : (t + 1) * P],
                                        rhs=qt_sb[:, h, sb * SBLK + sc0 : sb * SBLK + sc1],
                                        start=True,
                                        stop=True,
                                    )
                            ptile = pt_pool.tile([P, SBLK], bf16, tag="pt")
                            nc.scalar.activation(
                                ptile[:],
                                sc_ps[:],
                                AF.Exp,
                                bias=mask_sb[:, t : t + 1],
                                scale=1.0,
                            )
                            pts[t] = ptile
                        if h == 0 and sb == 0:
                            # V projection rides between the exps and the PVs
                            emit_v_pair(tp)
                        for j in range(2):
                            t = 2 * tp + j
                            for scnk in range(NSC):
                                nc.tensor.matmul(
                                    cacc_ap(scnk),
                                    lhsT=pts[t][:, scnk * P : (scnk + 1) * P],
                                    rhs=vt_sb[:, t, h, :],
                                    start=False,
                                    stop=(t == TCH - 1),
                                    skip_group_check=True,
                                )
                    # normalize by the ones-column denominator (per-partition
                    # broadcast) into cn_buf; head-odd flushes the head pair's
                    # [s,128] chunk to ct_sb via one DMA transpose
                    for scnk in range(NSC):
                        ca = cacc_ap(scnk)
                        recip = norm_pool.tile([P, 1], fp32, tag="recip")
                        nc.vector.reciprocal(recip[:], ca[:, D : D + 1])
                        nc.vector.tensor_scalar_mul(
                            cn_buf[:, sb, scnk, (h % 2) * D : (h % 2) * D + D],
                            ca[:, 0:D],
                            recip[:],
                        )
                        if h % 2 == 1:
                            nc.sync.dma_start_transpose(
                                ct_sb[:, pr, sb * SBLK + scnk * P : sb * SBLK + (scnk + 1) * P],
                                cn_buf[:, sb, scnk, :],
                            )
                            if h == NH - 1 and sb == NSB - 1:
                                # final s-block: this chunk's ct is complete
                                # for all heads the moment it's flushed, and
                                # the PE is otherwise idle during normalize
                                emit_o_chunk(
                                    sb * NSC + scnk,
                                    use_sc=(scnk % 2 == 1 and SBLK >= E),
                                )
                if not (sb == NSB - 1):
                    # queue this s-block's O chunks for injection at later
                    # head boundaries
                    o_ready.extend(sb * NSC + scnk for scnk in range(NSC))
            for i, schunk in enumerate(o_ready):
                emit_o_chunk(schunk, use_sc=(i % 2 == 1 and SBLK >= E))

    nc.compile()
    return nc


def _get_built(seq_len=S):
    if seq_len not in _BUILT:
        _BUILT[seq_len] = _build(seq_len)
    return _BUILT[seq_len]


def _host_prep(hidden_states, attention_mask, W_q, W_k, W_v, W_o, gate, seq_len=S):
    import ml_dtypes

    bf16 = ml_dtypes.bfloat16
    hs = np.asarray(hidden_states, dtype=np.float32)
    am = np.asarray(attention_mask, dtype=np.float32)
    W_q = np.asarray(W_q, dtype=np.float32)
    W_k = np.asarray(W_k, dtype=np.float32)
    W_v = np.asarray(W_v, dtype=np.float32)
    W_o = np.asarray(W_o, dtype=np.float32)
    gate = np.asarray(gate, dtype=np.float32)

    eff_gate = np.where(gate >= GATE_EPS, gate, 0.0)
    active = float(np.sum(gate > GATE_EPS))
    denom = max(1.0, active / H) if active > 0 else 1.0

    # 1/sqrt(D) softmax scale, and x0.5 compensating the duplicated-rows
    # K=128 score contraction on device
    scale = 0.5 / math.sqrt(D)
    # [H, E, D] -> [E, H*D] head-stacked
    wq_all = np.ascontiguousarray((W_q * scale).transpose(1, 0, 2).reshape(E, H * D)).astype(bf16)
    wk_all = np.ascontiguousarray(W_k.transpose(1, 0, 2).reshape(E, H * D)).astype(bf16)
    wv_all = np.ascontiguousarray(W_v.transpose(1, 0, 2).reshape(E, H * D)).astype(bf16)
    wo_scaled = (W_o * (eff_gate / denom)[:, None, None]).reshape(H * D, E).astype(bf16)

    in_maps = []
    for c in range(N_CORES):
        b = c // 4
        g = c % 4
        hd0 = g * NH * D
        xt_c = np.ascontiguousarray(hs[b, :seq_len].T).astype(bf16)  # [E, S]
        mask_c = np.ascontiguousarray(
            am[b, 0, 0, :seq_len].reshape(seq_len // P, P).T
        ).astype(np.float32)  # [128, TCH]
        in_maps.append(
            {
                "xt": xt_c,
                "wq": np.ascontiguousarray(wq_all[:, hd0 : hd0 + HDC]),
                "wk": np.ascontiguousarray(wk_all[:, hd0 : hd0 + HDC]),
                "wv": np.ascontiguousarray(wv_all[:, hd0 : hd0 + HDC]),
                "wo": np.ascontiguousarray(wo_scaled[hd0 : hd0 + HDC, :]),
                "mask": mask_c,
            }
        )
    return in_maps


LAST_RESULTS = None


def _ensure_ntff_hook():
    """Install the antenv.axon_hooks shim + ctypes NTFF hook if absent.

    The agent image's antenv package lacks axon_hooks, so bass_utils'
    trace=True path can't find the profile hook; recreate what
    trn_agent_boot would have registered.
    """
    import sys
    import types

    try:
        from antenv.axon_hooks import get_axon_ntff_profile_hook  # noqa: F401

        return
    except ImportError:
        pass
    mod = types.ModuleType("antenv.axon_hooks")
    state = {"hook": None}
    mod.set_axon_ntff_profile_hook = lambda h: state.__setitem__("hook", h)
    mod.get_axon_ntff_profile_hook = lambda: state["hook"]
    sys.modules["antenv.axon_hooks"] = mod
    try:
        import antenv

        antenv.axon_hooks = mod
    except ImportError:
        pass
    try:
        from trn_agent_boot.trn_boot import _ntff_profile_via_ctypes

        mod.set_axon_ntff_profile_hook(
            _ntff_profile_via_ctypes("/opt/axon/libaxon_pjrt.so")
        )
    except Exception:
        pass


def kernel(hidden_states, attention_mask, W_q, W_k, W_v, W_o, gate):
    global LAST_RESULTS
    from concourse.bass_utils import run_bass_kernel_spmd

    nc = _get_built(S)
    in_maps = _host_prep(hidden_states, attention_mask, W_q, W_k, W_v, W_o, gate)
    trace = bool(os.environ.get("BASS_TRACE"))
    if trace:
        _ensure_ntff_hook()
    res = run_bass_kernel_spmd(nc, in_maps, core_ids=list(range(N_CORES)), trace=trace)
    LAST_RESULTS = res

    out = np.zeros((B, S, E), dtype=np.float32)
    for c in range(N_CORES):
        out[c // 4] += np.asarray(res.results[c]["out"], dtype=np.float32)
    return out

